# revision 2
# baseline (speedup 1.0000x reference)
"""Trainium2 Bass kernel for nn_Nets_9337258902417 (gnn_message_passing).

Computes: elu(inputs @ scatter_nd(nonzero_ind, kernel_vector, [20000, 4096]) + bias)

Strategy:
  * Host: scatter-add the 2M sparse values into the dense [20000, 4096] kernel
    (np.bincount), fold `bias` in as an extra K-row paired with a ones-row in
    the inputs, pad K 20000 -> 20480 (= 160 tiles of 128), cast everything to
    fp16 (PSUM accumulation is fp32; rel-err ~1e-3), and pre-tile both
    operands so every device DMA is a fully contiguous block.
  * Device (SPMD x8, units sharded 512/core): tiled matmul
    out[2048, 512] = xT.T @ w with K on partitions.  K is split into 4
    "quarters" of 40 k-tiles so the weight quarter [128, 40, 512] fits in
    SBUF double-buffered; batch is split into 2 groups of 8 psum banks so all
    accumulation happens in PSUM (no SBUF partial sums).  ELU + bias fused in
    the epilogue (elu(x) = exp(min(x,0)) - 1 + max(x,0)).
  * Host: concatenate the 8 per-core [2048, 512] outputs along units.
"""

import numpy as np

BATCH = 2048
INPUT_DIM = 20000
UNITS = 4096
N_CORES = 8

KPAD = 20480          # 160 k-tiles of 128
NQ = 4                # K quarters
KT = 40               # k-tiles (of 128) per quarter
UPC = UNITS // N_CORES  # 512 units per core
MT = BATCH // 128     # 16 batch tiles of 128
NGROUP = 2            # batch-tile groups (8 psum banks each)
MPG = MT // NGROUP    # 8 batch tiles per group

_cache = {}


def _build_bass():
    import concourse.mybir as mybir
    import concourse.tile as tile
    from concourse import bacc

    F16 = mybir.dt.float16
    F32 = mybir.dt.float32

    nc = bacc.Bacc(
        "TRN2",
        target_bir_lowering=False,
        debug=False,
        enable_asserts=False,
        num_devices=N_CORES,
    )
    xq_d = nc.dram_tensor("xq", (NQ, MT, 128, KT, 128), F16, kind="ExternalInput")
    wq_d = nc.dram_tensor("wq", (NQ, 128, KT, UPC), F16, kind="ExternalInput")
    out_d = nc.dram_tensor("out", (BATCH, UPC), F32, kind="ExternalOutput")
    xq, wq, out = xq_d.ap(), wq_d.ap(), out_d.ap()

    with tile.TileContext(nc) as tc:
        with (
            tc.tile_pool(name="w", bufs=3) as wpool,
            tc.tile_pool(name="x", bufs=3) as xpool,
            tc.tile_pool(name="ep", bufs=3) as epool,
            tc.tile_pool(name="psum", bufs=8, space="PSUM") as pp,
        ):
            for g in range(NGROUP):
                psums = [pp.tile([128, UPC], F32, tag="ps", name=f"ps_{g}_{i}") for i in range(MPG)]
                for q in range(NQ):
                    wt = wpool.tile([128, KT, UPC], F16, tag="w")
                    nc.sync.dma_start(wt[:], wq[q])
                    for mi in range(MPG):
                        m = g * MPG + mi
                        xt = xpool.tile([128, KT, 128], F16, tag="x")
                        nc.sync.dma_start(xt[:], xq[q, m])
                        ps = psums[mi]
                        for k in range(KT):
                            nc.tensor.matmul(
                                ps,
                                lhsT=xt[:, k, :],
                                rhs=wt[:, k, :],
                                start=(q == 0 and k == 0),
                                stop=(q == NQ - 1 and k == KT - 1),
                            )
                        if q == NQ - 1:
                            # elu(x) = exp(min(x, 0)) - 1 + max(x, 0)
                            t = epool.tile([128, UPC], F32, tag="t")
                            nc.vector.tensor_scalar_min(t, ps, 0.0)
                            e = epool.tile([128, UPC], F32, tag="e")
                            nc.scalar.activation(
                                e, t, mybir.ActivationFunctionType.Exp
                            )
                            r = epool.tile([128, UPC], F32, tag="r")
                            nc.vector.tensor_scalar_max(r, ps, 0.0)
                            o = epool.tile([128, UPC], F32, tag="o")
                            nc.vector.scalar_tensor_tensor(
                                o, e, -1.0, r,
                                mybir.AluOpType.add, mybir.AluOpType.add,
                            )
                            nc.sync.dma_start(out[m * 128:(m + 1) * 128, :], o[:])
    nc.compile()
    return nc


def get_nc():
    if "nc" not in _cache:
        _cache["nc"] = _build_bass()
    return _cache["nc"]


def prepare_in_maps(inputs, kernel_vector, bias, nonzero_ind):
    """Host-side prep: scatter to dense, pad, fold bias, cast fp16, pre-tile."""
    rows = nonzero_ind[:, 0].astype(np.int64)
    cols = nonzero_ind[:, 1].astype(np.int64)
    flat = rows * UNITS + cols
    dense = np.bincount(flat, weights=kernel_vector, minlength=INPUT_DIM * UNITS)

    wpad = np.zeros((KPAD, UNITS), np.float16)
    wpad[:INPUT_DIM] = dense.reshape(INPUT_DIM, UNITS).astype(np.float16)
    wpad[INPUT_DIM] = bias.astype(np.float16)  # bias row, paired with ones in x

    xpad = np.zeros((KPAD, BATCH), np.float16)
    xpad[:INPUT_DIM] = inputs.T.astype(np.float16)
    xpad[INPUT_DIM] = np.float16(1.0)

    # [KPAD, BATCH] -> [q, m, p, kt, f]: row = q*5120 + kt*128 + p, col = m*128 + f
    xq = np.ascontiguousarray(
        xpad.reshape(NQ, KT, 128, MT, 128).transpose(0, 3, 2, 1, 4)
    )
    in_maps = []
    for c in range(N_CORES):
        wc = np.ascontiguousarray(
            wpad[:, c * UPC:(c + 1) * UPC]
            .reshape(NQ, KT, 128, UPC)
            .transpose(0, 2, 1, 3)
        )
        in_maps.append({"xq": xq, "wq": wc})
    return in_maps


def run_device(in_maps, trace=False):
    import concourse.bass_utils as bass_utils

    nc = get_nc()
    res = bass_utils.run_bass_kernel_spmd(
        nc, in_maps, core_ids=list(range(N_CORES)), trace=trace
    )
    return res


def kernel(inputs, kernel_vector, bias, nonzero_ind):
    in_maps = prepare_in_maps(inputs, kernel_vector, bias, nonzero_ind)
    res = run_device(in_maps, trace=False)
    outs = [r["out"] for r in res.results]
    return np.ascontiguousarray(np.concatenate(outs, axis=1), dtype=np.float32)


# revision 3
# speedup vs baseline: 1.2373x; 1.2373x over previous
"""Trainium2 Bass kernel for nn_Nets_9337258902417 (gnn_message_passing).

Computes: elu(inputs @ scatter_nd(nonzero_ind, kernel_vector, [20000, 4096]) + bias)

Strategy:
  * Host: scatter-add the 2M sparse values into the dense [20000, 4096] kernel
    (np.bincount), fold `bias` in as an extra K-row paired with a ones-row in
    the inputs, pad K 20000 -> 20480 (= 160 tiles of 128), cast everything to
    bf16 (PSUM accumulation is fp32; measured rel-err ~2.4e-3), and pre-tile
    both operands so every device DMA is a fully contiguous block.
  * Device (SPMD x8, units sharded 512/core): tiled matmul
    out[2048, 512] = xT.T @ w with the contraction (K) on partitions.  K is
    split into 4 "quarters" of 40 k-tiles so a weight quarter [128, 40, 512]
    fits in SBUF multi-buffered; the 16 batch tiles are split into 2 groups
    of 8 so each group's accumulation lives entirely in the 8 PSUM banks
    across all four K-quarters (weights are streamed twice - this kernel is
    TensorEngine-bound, so the extra DMA is hidden).  ELU + bias fused in the
    epilogue: elu(x) = exp(min(x,0)) - 1 + max(x,0).
  * Host: concatenate the 8 per-core [2048, 512] outputs along units.

Measured (loop-amplified on HW): ~575 us/core device time; PE streaming
floor for this shape is 2560 matmuls x 213 ns = 546 us.
"""

import numpy as np

BATCH = 2048
INPUT_DIM = 20000
UNITS = 4096
N_CORES = 8

KPAD = 20480          # 160 k-tiles of 128
NQ = 4                # K quarters
KT = 40               # k-tiles (of 128) per quarter
UPC = UNITS // N_CORES  # 512 units per core
MT = BATCH // 128     # 16 batch tiles of 128
NGROUP = 2            # batch-tile groups (8 psum banks each)
MPG = MT // NGROUP    # 8 batch tiles per group

_cache = {}


def _bf16_dtype():
    import ml_dtypes

    return np.dtype(ml_dtypes.bfloat16)


def _to_bf16(a):
    """float32 ndarray -> bfloat16 via round-to-nearest-even bit twiddling."""
    u = np.ascontiguousarray(a, dtype=np.float32).view(np.uint32)
    r = (u + np.uint32(0x7FFF) + ((u >> np.uint32(16)) & np.uint32(1))) >> np.uint32(16)
    return r.astype(np.uint16).view(_bf16_dtype())


def _build_bass():
    import concourse.mybir as mybir
    import concourse.tile as tile
    from concourse import bacc

    F16 = mybir.dt.bfloat16
    F32 = mybir.dt.float32

    nc = bacc.Bacc(
        "TRN2",
        target_bir_lowering=False,
        debug=False,
        enable_asserts=False,
        num_devices=N_CORES,
    )
    xq_d = nc.dram_tensor("xq", (NQ, MT, 128, KT, 128), F16, kind="ExternalInput")
    wq_d = nc.dram_tensor("wq", (NQ, 128, KT, UPC), F16, kind="ExternalInput")
    out_d = nc.dram_tensor("out", (BATCH, UPC), F32, kind="ExternalOutput")
    xq, wq, out = xq_d.ap(), wq_d.ap(), out_d.ap()

    with tile.TileContext(nc) as tc:
        with (
            tc.tile_pool(name="w", bufs=3) as wpool,
            tc.tile_pool(name="x", bufs=3) as xpool,
            tc.tile_pool(name="ep", bufs=3) as epool,
            tc.tile_pool(name="psum", bufs=8, space="PSUM") as pp,
        ):
            for g in range(NGROUP):
                psums = [pp.tile([128, UPC], F32, tag="ps", name=f"ps_{g}_{i}")
                         for i in range(MPG)]
                for q in range(NQ):
                    wt = wpool.tile([128, KT, UPC], F16, tag="w", name="wt")
                    nc.sync.dma_start(wt[:], wq[q])
                    for mi in range(MPG):
                        m = g * MPG + mi
                        xt = xpool.tile([128, KT, 128], F16, tag="x", name="xt")
                        nc.sync.dma_start(xt[:], xq[q, m])
                        ps = psums[mi]
                        for k in range(KT):
                            nc.tensor.matmul(
                                ps,
                                lhsT=xt[:, k, :],
                                rhs=wt[:, k, :],
                                start=(q == 0 and k == 0),
                                stop=(q == NQ - 1 and k == KT - 1),
                            )
                        if q == NQ - 1:
                            # elu(x) = exp(min(x, 0)) - 1 + max(x, 0)
                            t = epool.tile([128, UPC], F32, tag="t", name="t")
                            nc.vector.tensor_scalar_min(t, ps, 0.0)
                            e = epool.tile([128, UPC], F32, tag="e", name="e")
                            nc.scalar.activation(
                                e, t, mybir.ActivationFunctionType.Exp
                            )
                            r = epool.tile([128, UPC], F32, tag="r", name="r")
                            nc.vector.tensor_scalar_max(r, ps, 0.0)
                            o = epool.tile([128, UPC], F32, tag="o", name="o")
                            nc.vector.scalar_tensor_tensor(
                                o, e, -1.0, r,
                                mybir.AluOpType.add, mybir.AluOpType.add,
                            )
                            nc.sync.dma_start(out[m * 128:(m + 1) * 128, :], o[:])
    nc.compile()
    return nc


def get_nc():
    if "nc" not in _cache:
        _cache["nc"] = _build_bass()
    return _cache["nc"]


def prepare_in_maps(inputs, kernel_vector, bias, nonzero_ind):
    """Host-side prep: scatter to dense, pad, fold bias, cast bf16, pre-tile."""
    bf16 = _bf16_dtype()
    rows = nonzero_ind[:, 0].astype(np.int64)
    cols = nonzero_ind[:, 1].astype(np.int64)
    flat = rows * UNITS + cols
    dense = np.bincount(flat, weights=kernel_vector, minlength=INPUT_DIM * UNITS)

    wpad = np.zeros((KPAD, UNITS), bf16)
    wpad[:INPUT_DIM] = _to_bf16(dense.astype(np.float32)).reshape(INPUT_DIM, UNITS)
    wpad[INPUT_DIM] = _to_bf16(np.asarray(bias, dtype=np.float32))

    xpad = np.zeros((KPAD, BATCH), bf16)
    xpad[:INPUT_DIM] = _to_bf16(np.asarray(inputs, dtype=np.float32)).T
    xpad[INPUT_DIM] = np.float32(1.0)  # ones row pairs with the bias row in w

    # [KPAD, BATCH] -> [q, m, p, kt, f]: row = q*5120 + kt*128 + p, col = m*128 + f
    xq = np.ascontiguousarray(
        xpad.reshape(NQ, KT, 128, MT, 128).transpose(0, 3, 2, 1, 4)
    )
    in_maps = []
    for c in range(N_CORES):
        wc = np.ascontiguousarray(
            wpad[:, c * UPC:(c + 1) * UPC]
            .reshape(NQ, KT, 128, UPC)
            .transpose(0, 2, 1, 3)
        )
        in_maps.append({"xq": xq, "wq": wc})
    return in_maps


def run_device(in_maps, trace=False):
    import concourse.bass_utils as bass_utils

    nc = get_nc()
    res = bass_utils.run_bass_kernel_spmd(
        nc, in_maps, core_ids=list(range(N_CORES)), trace=trace
    )
    return res


def kernel(inputs, kernel_vector, bias, nonzero_ind):
    in_maps = prepare_in_maps(inputs, kernel_vector, bias, nonzero_ind)
    res = run_device(in_maps, trace=False)
    outs = [r["out"] for r in res.results]
    return np.ascontiguousarray(np.concatenate(outs, axis=1), dtype=np.float32)
